# revision 21
# baseline (speedup 1.0000x reference)
"""Trainium2 Bass kernel for InvertedResidual (ShuffleNetV2 stride-1 unit).

Reference computation (per image, NCHW, C=232):
    x1, x2 = split(x, 116)
    h  = prelu(bn1(conv1x1(x2, w2)), a1)
    h2 = bn2(dwconv3x3(h, wdw))
    y  = prelu(bn3(conv1x1(h2, w3)), a2)
    out = channel_shuffle(concat(x1, y))   # out[2j]=x1[j], out[2j+1]=y[j]

Device mapping (per core, 8 images, data-parallel over batch=64 on 8 cores):
  - channels (116) on SBUF partitions, pixels (56*56=3136) on free dim
  - HBM transport is bf16 (host casts x to bf16 / result back to f32);
    rel-err budget is 2e-2, bf16 transport costs ~2e-3. Halves HBM traffic.
  - big DMAs are issued on the gpsimd (Pool/SWDGE) path: SWDGE sprays
    descriptors over all 16 SDMA engines, vs only 4 for qSPDynamicHW.
  - conv1x1 #1 runs in bf16 (weights bf16, rhs straight from the DMA'd
    bf16 x2 tile), BN1 folded, PReLU #1 via ScalarE -> padded f32r h1.
  - dw3x3 + BN2 + conv1x1 #2 + BN3 fused: for each of the 9 taps (dy,dx),
    W_t = (w3*s3[:,None]) @ diag(wdw[:,dy,dx]*s2)  -> 9 dense 116-wide
    f32r matmuls per chunk accumulating in PSUM over spatially shifted
    views of h1 (taps-inner: per-chunk groups keep the PE stream dense
    and let chunks pipeline across PSUM banks).  Contraction is padded
    to k=118 with a constant-ones partition so the BN3/conv2 bias
    (t3 + W3'@t2) rides the center tap's matmul for free.
  - PReLU #2 on the otherwise-idle VectorE: with bias already in PSUM,
    prelu(u) = max(a*u, u) is one scalar_tensor_tensor op -> bf16 out.
  - x1 passthrough: DRAM->DRAM DMA into interleaved output channels
"""

import numpy as np

EPS = 1e-5
BPC = 8            # images per core
NCORES = 8
BF = 116           # branch features
KP = 118           # padded contraction: 116 ch + ones row (bias) + zero row
H = W = 56
HW = H * W         # 3136
RPC = 8            # output rows per chunk
NCH = H // RPC     # 7 chunks per image
CHW = RPC * W      # 448 columns per matmul (<=512 fp32 PSUM bank)

# tap order: center first so the start=True matmul covers all pixels
TAP_ORDER = [(1, 1)] + [(dy, dx) for dy in range(3) for dx in range(3)
                        if (dy, dx) != (1, 1)]

_CACHE = {}


def _build(alpha1: float, alpha2: float, reps: int = 1, variant: str = "full"):
    import concourse.mybir as mybir
    import concourse.tile as tile
    from concourse import bacc

    f32 = mybir.dt.float32
    f32r = mybir.dt.float32r
    bf16 = mybir.dt.bfloat16
    PRELU = mybir.ActivationFunctionType.Prelu
    COPY = mybir.ActivationFunctionType.Copy
    MULT = mybir.AluOpType.mult
    MAX = mybir.AluOpType.max

    nc = bacc.Bacc("TRN2", target_bir_lowering=False, debug=False)
    # x viewed as [b, half, 116, HW]; out viewed as [b, 116, 2, HW] so that
    # out[b, j, 0] is channel 2j (=x1[j]) and out[b, j, 1] is channel 2j+1 (=y[j])
    x = nc.dram_tensor("x", [BPC, 2, BF, HW], bf16, kind="ExternalInput")
    w1 = nc.dram_tensor("w1", [BF, BF], bf16, kind="ExternalInput")
    wts = nc.dram_tensor("wts", [KP, 9 * BF], f32r, kind="ExternalInput")
    bias = nc.dram_tensor("bias", [BF, 2], f32, kind="ExternalInput")
    ones = nc.dram_tensor("ones", [2, (W + 2) * (W + 2)], f32r,
                          kind="ExternalInput")
    out = nc.dram_tensor("out", [BPC, BF, 2, HW], bf16, kind="ExternalOutput")

    PW = W + 2  # 58

    with tile.TileContext(nc) as tc:
        with (
            tc.tile_pool(name="const", bufs=1) as constp,
            tc.tile_pool(name="xin", bufs=3) as xinp,
            tc.tile_pool(name="hout", bufs=2) as houtp,
            tc.tile_pool(name="ps1", bufs=2, space="PSUM") as ps1p,
            tc.tile_pool(name="ps2", bufs=4, space="PSUM") as ps2p,
        ):
            # consts ride the sync/HWDGE queue: SWDGE completions batch at
            # backlog-drain time, so the Pool queue is kept for the big
            # paced transfers and the first x2 load completes fast.
            w1sb = constp.tile([BF, BF], bf16)
            nc.sync.dma_start(w1sb[:], w1[:, :])
            bsb = constp.tile([BF, 2], f32)
            nc.sync.dma_start(bsb[:], bias[:, :])
            wsb = constp.tile([KP, 9 * BF], f32r)
            nc.sync.dma_start(wsb[:], wts[:, :])

            def wslice(i):
                return wsb[:, i * BF:(i + 1) * BF]

            # Persistent double-buffered padded h1 [58x58] with zero borders.
            # Zero-padding makes all 9 taps full-coverage; borders zeroed
            # once, per-image prelu1 writes only the interior.  Partition
            # 116 is constant 1.0 (bias row), 117 constant 0.
            h1bufs = []
            for i in range(2):
                t = constp.tile([KP, PW * PW], f32r, tag=f"h1_{i}")
                tv = t[:].rearrange("p (r c) -> p r c", c=PW)
                nc.scalar.activation(tv[0:BF, 0, :], wsb[0:BF, 0:PW], COPY,
                                     scale=0.0)
                nc.scalar.activation(tv[0:BF, PW - 1, :], wsb[0:BF, 0:PW],
                                     COPY, scale=0.0)
                nc.scalar.activation(tv[0:BF, 1:PW - 1, 0], wsb[0:BF, 0:W],
                                     COPY, scale=0.0)
                nc.scalar.activation(tv[0:BF, 1:PW - 1, PW - 1],
                                     wsb[0:BF, 0:W], COPY, scale=0.0)
                nc.sync.dma_start(t[BF:KP, :], ones[:, :])
                h1bufs.append(t)

            for rep in range(reps):
              for b in range(BPC):
                # x2 load paced by the 3-deep pool: at most ~3 loads queue
                # ahead of compute, so the first image's load isn't stuck
                # behind a large SWDGE backlog (a DMA only completes when
                # all 16 engines drain its descriptors from their rings).
                x2t = xinp.tile([BF, HW], bf16, tag="x2t")
                nc.gpsimd.dma_start(x2t[:], x[b, 1, :, :])

                # conv1x1 #1 (bf16) + BN1 + PReLU -> padded h1 interior
                h1t = h1bufs[b % 2]
                h1v = h1t[:].rearrange("p (r c) -> p r c", c=PW)
                for j in range(NCH):
                    ps1 = ps1p.tile([BF, CHW], f32)
                    nc.tensor.matmul(
                        ps1[:], w1sb[:],
                        x2t[:, j * CHW:(j + 1) * CHW],
                        start=True, stop=True)
                    nc.scalar.activation(
                        h1v[0:BF, 1 + j * RPC:1 + (j + 1) * RPC, 1:1 + W],
                        ps1[:], PRELU,
                        bias=bsb[:, 0:1], scale=1.0, alpha=alpha1)

                # passthrough half: even output channels, straight from HBM;
                # issued mid-body so it spreads across the run, not a tail,
                # and never sits ahead of a compute-critical x2 load.
                nc.gpsimd.dma_start(out[b, :, 0, :], x[b, 0, :, :])

                # fused dw3x3+BN2+conv1x1#2+BN3+bias (9 matmuls, k=118) then
                # PReLU#2 (bias already accumulated in PSUM) -> bf16 hot
                hot = houtp.tile([BF, HW], bf16)
                for j in range(NCH):
                    ps2 = ps2p.tile([BF, CHW], f32)
                    for ti, (dy, dx) in enumerate(TAP_ORDER):
                        nc.tensor.matmul(
                            ps2[:],
                            wslice(ti),
                            h1v[:, j * RPC + dy:j * RPC + dy + RPC,
                                dx:dx + W],
                            start=(ti == 0), stop=(ti == len(TAP_ORDER) - 1))
                    nc.scalar.activation(
                        hot[:, j * CHW:(j + 1) * CHW], ps2[:], PRELU,
                        bias=0.0, scale=1.0, alpha=alpha2)

                nc.gpsimd.dma_start(out[b, :, 1, :], hot[:])
    if not nc.is_finalized():
        nc.finalize()
    return nc


def _prep_host(w2, bn1_g, bn1_b, bn1_m, bn1_v, wdw, bn2_g, bn2_b, bn2_m,
               bn2_v, w3, bn3_g, bn3_b, bn3_m, bn3_v):
    import ml_dtypes

    s1 = bn1_g / np.sqrt(bn1_v + EPS)
    t1 = bn1_b - bn1_m * s1
    s2 = bn2_g / np.sqrt(bn2_v + EPS)
    t2 = bn2_b - bn2_m * s2
    s3 = bn3_g / np.sqrt(bn3_v + EPS)
    t3 = bn3_b - bn3_m * s3
    w3p = w3 * s3[:, None]                  # [o,c] BN3-folded conv2 weights
    wdwp = wdw[:, 0] * s2[:, None, None]    # [c,3,3] BN2-folded dw weights

    w1 = np.ascontiguousarray((w2 * s1[:, None]).T).astype(ml_dtypes.bfloat16)
    wts = np.zeros((KP, 9 * BF), np.float32)
    for ti, (dy, dx) in enumerate(TAP_ORDER):
        wts[0:BF, ti * BF:(ti + 1) * BF] = w3p.T * wdwp[:, dy, dx][:, None]
    # conv2/BN3 bias rides the center tap via the constant-ones partition
    wts[BF, 0:BF] = t3 + w3p @ t2

    bias = np.empty((BF, 2), np.float32)
    bias[:, 0] = t1
    bias[:, 1] = 0.0
    return w1, np.ascontiguousarray(wts), np.ascontiguousarray(bias)


def _run(inputs, trace=False, trace_kwargs=None, reps=1):
    import ml_dtypes
    from concourse.bass_utils import run_bass_kernel_spmd

    a1 = float(np.asarray(inputs["alpha1"]).reshape(-1)[0])
    a2 = float(np.asarray(inputs["alpha2"]).reshape(-1)[0])
    key = (a1, a2, reps)
    if key not in _CACHE:
        _CACHE[key] = _build(a1, a2, reps)
    nc = _CACHE[key]

    w1, wts, bias = _prep_host(*[np.asarray(inputs[k], np.float32) for k in (
        "w2", "bn1_g", "bn1_b", "bn1_m", "bn1_v", "wdw", "bn2_g", "bn2_b",
        "bn2_m", "bn2_v", "w3", "bn3_g", "bn3_b", "bn3_m", "bn3_v")])

    x = np.asarray(inputs["x"], np.float32).astype(ml_dtypes.bfloat16)
    x = np.ascontiguousarray(x)
    B = x.shape[0]
    assert B == BPC * NCORES
    xr = x.reshape(NCORES, BPC, 2, BF, HW)

    ones = np.zeros((2, (W + 2) * (W + 2)), np.float32)
    ones[0] = 1.0
    in_maps = [
        {"x": np.ascontiguousarray(xr[c]), "w1": w1, "wts": wts, "bias": bias,
         "ones": ones}
        for c in range(NCORES)
    ]
    kw = {}
    if trace:
        kw["trace"] = True
        kw.update(trace_kwargs or {})
    res = run_bass_kernel_spmd(nc, in_maps, core_ids=list(range(NCORES)), **kw)
    outs = np.stack([res.results[c]["out"] for c in range(NCORES)])
    # [cores, bpc, 116, 2, HW] -> [64, 232, 56, 56]
    full = outs.reshape(B, 232, H, W).astype(np.float32)
    return full, res


def kernel(**inputs) -> np.ndarray:
    full, _ = _run(inputs, trace=False)
    return full


# revision 22
# speedup vs baseline: 1.0106x; 1.0106x over previous
"""Trainium2 Bass kernel for InvertedResidual (ShuffleNetV2 stride-1 unit).

Reference computation (per image, NCHW, C=232):
    x1, x2 = split(x, 116)
    h  = prelu(bn1(conv1x1(x2, w2)), a1)
    h2 = bn2(dwconv3x3(h, wdw))
    y  = prelu(bn3(conv1x1(h2, w3)), a2)
    out = channel_shuffle(concat(x1, y))   # out[2j]=x1[j], out[2j+1]=y[j]

Device mapping (per core, 8 images, data-parallel over batch=64 on 8 cores):
  - channels (116) on SBUF partitions, pixels (56*56=3136) on free dim
  - HBM transport is bf16 (host casts x to bf16 / result back to f32);
    rel-err budget is 2e-2, bf16 transport costs ~2e-3. Halves HBM traffic.
  - big DMAs are issued on the gpsimd (Pool/SWDGE) path: SWDGE sprays
    descriptors over all 16 SDMA engines, vs only 4 for qSPDynamicHW.
  - conv1x1 #1 runs in bf16 (weights bf16, rhs straight from the DMA'd
    bf16 x2 tile), BN1 folded, PReLU #1 via ScalarE -> padded f32r h1.
  - dw3x3 + BN2 + conv1x1 #2 + BN3 fused: for each of the 9 taps (dy,dx),
    W_t = (w3*s3[:,None]) @ diag(wdw[:,dy,dx]*s2)  -> 9 dense 116-wide
    f32r matmuls per chunk accumulating in PSUM over spatially shifted
    views of h1 (taps-inner: per-chunk groups keep the PE stream dense
    and let chunks pipeline across PSUM banks).  Contraction is padded
    to k=118 with a constant-ones partition so the BN3/conv2 bias
    (t3 + W3'@t2) rides the center tap's matmul for free.
  - PReLU #2 on the otherwise-idle VectorE: with bias already in PSUM,
    prelu(u) = max(a*u, u) is one scalar_tensor_tensor op -> bf16 out.
  - x1 passthrough: DRAM->DRAM DMA into interleaved output channels
"""

import numpy as np

EPS = 1e-5
BPC = 8            # images per core
NCORES = 8
BF = 116           # branch features
KP = 118           # padded contraction: 116 ch + ones row (bias) + zero row
H = W = 56
HW = H * W         # 3136
RPC = 8            # output rows per chunk
NCH = H // RPC     # 7 chunks per image
CHW = RPC * W      # 448 columns per matmul (<=512 fp32 PSUM bank)

# tap order: center first so the start=True matmul covers all pixels
TAP_ORDER = [(1, 1)] + [(dy, dx) for dy in range(3) for dx in range(3)
                        if (dy, dx) != (1, 1)]

_CACHE = {}


def _build(alpha1: float, alpha2: float, reps: int = 1, variant: str = "full"):
    import concourse.mybir as mybir
    import concourse.tile as tile
    from concourse import bacc

    f32 = mybir.dt.float32
    f32r = mybir.dt.float32r
    bf16 = mybir.dt.bfloat16
    PRELU = mybir.ActivationFunctionType.Prelu
    COPY = mybir.ActivationFunctionType.Copy
    MULT = mybir.AluOpType.mult
    MAX = mybir.AluOpType.max

    nc = bacc.Bacc("TRN2", target_bir_lowering=False, debug=False)
    # x viewed as [b, half, 116, HW]; out viewed as [b, 116, 2, HW] so that
    # out[b, j, 0] is channel 2j (=x1[j]) and out[b, j, 1] is channel 2j+1 (=y[j])
    x = nc.dram_tensor("x", [BPC, 2, BF, HW], bf16, kind="ExternalInput")
    w1 = nc.dram_tensor("w1", [BF, BF], bf16, kind="ExternalInput")
    wts = nc.dram_tensor("wts", [KP, 9 * BF], f32r, kind="ExternalInput")
    bias = nc.dram_tensor("bias", [BF, 2], f32, kind="ExternalInput")
    ones = nc.dram_tensor("ones", [2, (W + 2) * (W + 2)], f32r,
                          kind="ExternalInput")
    out = nc.dram_tensor("out", [BPC, BF, 2, HW], bf16, kind="ExternalOutput")

    PW = W + 2  # 58

    with tile.TileContext(nc) as tc:
        with (
            tc.tile_pool(name="const", bufs=1) as constp,
            tc.tile_pool(name="xin", bufs=2) as xinp,
            tc.tile_pool(name="hout", bufs=2) as houtp,
            tc.tile_pool(name="ps1", bufs=2, space="PSUM") as ps1p,
            tc.tile_pool(name="ps2", bufs=4, space="PSUM") as ps2p,
        ):
            # consts ride the sync/HWDGE queue: SWDGE completions batch at
            # backlog-drain time, so the Pool queue is kept for the big
            # paced transfers and the first x2 load completes fast.
            w1sb = constp.tile([BF, BF], bf16)
            nc.sync.dma_start(w1sb[:], w1[:, :])
            bsb = constp.tile([BF, 2], f32)
            nc.sync.dma_start(bsb[:], bias[:, :])
            wsb = constp.tile([KP, 9 * BF], f32r)
            nc.sync.dma_start(wsb[:], wts[:, :])

            def wslice(i):
                return wsb[:, i * BF:(i + 1) * BF]

            # Persistent double-buffered padded h1 [58x58] with zero borders.
            # Zero-padding makes all 9 taps full-coverage; borders zeroed
            # once, per-image prelu1 writes only the interior.  Partition
            # 116 is constant 1.0 (bias row), 117 constant 0.
            h1bufs = []
            for i in range(2):
                t = constp.tile([KP, PW * PW], f32r, tag=f"h1_{i}")
                tv = t[:].rearrange("p (r c) -> p r c", c=PW)
                nc.scalar.activation(tv[0:BF, 0, :], wsb[0:BF, 0:PW], COPY,
                                     scale=0.0)
                nc.scalar.activation(tv[0:BF, PW - 1, :], wsb[0:BF, 0:PW],
                                     COPY, scale=0.0)
                nc.scalar.activation(tv[0:BF, 1:PW - 1, 0], wsb[0:BF, 0:W],
                                     COPY, scale=0.0)
                nc.scalar.activation(tv[0:BF, 1:PW - 1, PW - 1],
                                     wsb[0:BF, 0:W], COPY, scale=0.0)
                nc.sync.dma_start(t[BF:KP, :], ones[:, :])
                h1bufs.append(t)

            for rep in range(reps):
              for b in range(BPC):
                # x2 load paced by the 3-deep pool: at most ~3 loads queue
                # ahead of compute, so the first image's load isn't stuck
                # behind a large SWDGE backlog (a DMA only completes when
                # all 16 engines drain its descriptors from their rings).
                x2t = xinp.tile([BF, HW], bf16, tag="x2t")
                nc.gpsimd.dma_start(x2t[:], x[b, 1, :, :])

                # conv1x1 #1 (bf16) + BN1 + PReLU -> padded h1 interior
                h1t = h1bufs[b % 2]
                h1v = h1t[:].rearrange("p (r c) -> p r c", c=PW)
                for j in range(NCH):
                    ps1 = ps1p.tile([BF, CHW], f32)
                    nc.tensor.matmul(
                        ps1[:], w1sb[:],
                        x2t[:, j * CHW:(j + 1) * CHW],
                        start=True, stop=True)
                    nc.scalar.activation(
                        h1v[0:BF, 1 + j * RPC:1 + (j + 1) * RPC, 1:1 + W],
                        ps1[:], PRELU,
                        bias=bsb[:, 0:1], scale=1.0, alpha=alpha1)

                # passthrough half: even output channels, straight from HBM.
                # Rides the FIFO sync/HWDGE queue: SWDGE serves its backlog
                # round-robin, so any Pool-queue sibling would delay the
                # compute-critical x2 loads there.
                nc.sync.dma_start(out[b, :, 0, :], x[b, 0, :, :])

                # fused dw3x3+BN2+conv1x1#2+BN3+bias (9 matmuls, k=118) then
                # PReLU#2 (bias already accumulated in PSUM) -> bf16 hot
                hot = houtp.tile([BF, HW], bf16)
                for j in range(NCH):
                    ps2 = ps2p.tile([BF, CHW], f32)
                    for ti, (dy, dx) in enumerate(TAP_ORDER):
                        nc.tensor.matmul(
                            ps2[:],
                            wslice(ti),
                            h1v[:, j * RPC + dy:j * RPC + dy + RPC,
                                dx:dx + W],
                            start=(ti == 0), stop=(ti == len(TAP_ORDER) - 1))
                    nc.scalar.activation(
                        hot[:, j * CHW:(j + 1) * CHW], ps2[:], PRELU,
                        bias=0.0, scale=1.0, alpha=alpha2)

                nc.gpsimd.dma_start(out[b, :, 1, :], hot[:])
    if not nc.is_finalized():
        nc.finalize()
    return nc


def _prep_host(w2, bn1_g, bn1_b, bn1_m, bn1_v, wdw, bn2_g, bn2_b, bn2_m,
               bn2_v, w3, bn3_g, bn3_b, bn3_m, bn3_v):
    import ml_dtypes

    s1 = bn1_g / np.sqrt(bn1_v + EPS)
    t1 = bn1_b - bn1_m * s1
    s2 = bn2_g / np.sqrt(bn2_v + EPS)
    t2 = bn2_b - bn2_m * s2
    s3 = bn3_g / np.sqrt(bn3_v + EPS)
    t3 = bn3_b - bn3_m * s3
    w3p = w3 * s3[:, None]                  # [o,c] BN3-folded conv2 weights
    wdwp = wdw[:, 0] * s2[:, None, None]    # [c,3,3] BN2-folded dw weights

    w1 = np.ascontiguousarray((w2 * s1[:, None]).T).astype(ml_dtypes.bfloat16)
    wts = np.zeros((KP, 9 * BF), np.float32)
    for ti, (dy, dx) in enumerate(TAP_ORDER):
        wts[0:BF, ti * BF:(ti + 1) * BF] = w3p.T * wdwp[:, dy, dx][:, None]
    # conv2/BN3 bias rides the center tap via the constant-ones partition
    wts[BF, 0:BF] = t3 + w3p @ t2

    bias = np.empty((BF, 2), np.float32)
    bias[:, 0] = t1
    bias[:, 1] = 0.0
    return w1, np.ascontiguousarray(wts), np.ascontiguousarray(bias)


def _run(inputs, trace=False, trace_kwargs=None, reps=1):
    import ml_dtypes
    from concourse.bass_utils import run_bass_kernel_spmd

    a1 = float(np.asarray(inputs["alpha1"]).reshape(-1)[0])
    a2 = float(np.asarray(inputs["alpha2"]).reshape(-1)[0])
    key = (a1, a2, reps)
    if key not in _CACHE:
        _CACHE[key] = _build(a1, a2, reps)
    nc = _CACHE[key]

    w1, wts, bias = _prep_host(*[np.asarray(inputs[k], np.float32) for k in (
        "w2", "bn1_g", "bn1_b", "bn1_m", "bn1_v", "wdw", "bn2_g", "bn2_b",
        "bn2_m", "bn2_v", "w3", "bn3_g", "bn3_b", "bn3_m", "bn3_v")])

    x = np.asarray(inputs["x"], np.float32).astype(ml_dtypes.bfloat16)
    x = np.ascontiguousarray(x)
    B = x.shape[0]
    assert B == BPC * NCORES
    xr = x.reshape(NCORES, BPC, 2, BF, HW)

    ones = np.zeros((2, (W + 2) * (W + 2)), np.float32)
    ones[0] = 1.0
    in_maps = [
        {"x": np.ascontiguousarray(xr[c]), "w1": w1, "wts": wts, "bias": bias,
         "ones": ones}
        for c in range(NCORES)
    ]
    kw = {}
    if trace:
        kw["trace"] = True
        kw.update(trace_kwargs or {})
    res = run_bass_kernel_spmd(nc, in_maps, core_ids=list(range(NCORES)), **kw)
    outs = np.stack([res.results[c]["out"] for c in range(NCORES)])
    # [cores, bpc, 116, 2, HW] -> [64, 232, 56, 56]
    full = outs.reshape(B, 232, H, W).astype(np.float32)
    return full, res


def kernel(**inputs) -> np.ndarray:
    full, _ = _run(inputs, trace=False)
    return full
